# revision 25
# baseline (speedup 1.0000x reference)
"""MoE layer (top-2 routing, 8 experts) on 8 Trainium2 NeuronCores.

Strategy (expert parallelism, per sharding hint):
  - Host computes the gate (logits -> top-k -> softmax) and routes tokens:
    expert e's tokens are gathered, padded to a common capacity C, and sent
    to core e.  This is the host-side equivalent of the "all-to-all tokens
    by expert assignment" step.
  - Core e runs the expert FFN for its tokens:
        yT = (gelu(x @ W1[e] + b1[e]) @ W2[e] + b2[e])^T
    entirely on-device in a transpose-free layout:
      mm1:  h^T[f,c] = sum_k W1_blk[k,f].T @ x^T[k,c]   (W1 stationary)
      gelu: ACT engine, exact (erf) Gelu, bias b1 fused
      mm2:  y^T[d,c] = sum_f W2_blk[f,d].T @ h^T[f,c]   (W2 stationary)
    Activations/weights are bf16 (full PE rate), accumulation fp32 in PSUM.
  - Both expert weight stacks (16.8 MB bf16) stay RESIDENT in SBUF: they are
    DMA'd exactly once, laid out per-partition-contiguous in DRAM so the
    load runs at full HBM bandwidth, split across the two hardware DGE
    queues (sync + scalar) and paced so arrival stays ahead of the first
    chunk's consumption.  Token chunks then stream through with zero
    steady-state weight traffic, keeping the PE at its bf16 roofline.
  - Host scatters y back, scaled by the gate weights, and sums the top-k
    expert contributions per token.

Hardcoded problem shape: x [4, 2048, 1024], E=8 experts, D=1024, F=4096.
"""

import numpy as np
import ml_dtypes

import concourse.bass as bass
import concourse.mybir as mybir
import concourse.tile as tile
from concourse import bacc
from concourse.bass_utils import run_bass_kernel_spmd

D = 1024
F = 4096
E = 8
KD = D // 128   # 8 k-tiles over D
KF = F // 128   # 32 k-tiles over F
NT = 512        # max token chunk width (one PSUM bank of fp32)

_KERNEL_CACHE = {}


def _chunks(C):
    """Token chunks: one full 512 first (covers the one-time weight load),
    then the rest in equal widths (multiples of 8, each <= 512).

    Chunks narrower than ~256 run LDWEIGHTS-bound on the PE (the stationary
    load no longer hides under the matmul), so avoid a tiny remainder chunk.
    """
    if C <= NT:
        return [(0, C)]
    first = 256
    k = -(-(C - first) // NT)
    base = ((C - first) // k) & ~7
    if base < 256:
        # fall back to plain equal split
        first = 0
        k = -(-C // NT)
        base = (C // k) & ~7
    rest = C - first
    rem = rest - base * k
    assert rem % 8 == 0
    widths = ([first] if first else []) + [base + 8 * (i < rem // 8) for i in range(k)]
    out, c0 = [], 0
    for w in widths:
        out.append((c0, w))
        c0 += w
    assert c0 == C
    return out


def _build_kernel(C: int):
    """Per-core expert-FFN kernel for capacity C (multiple of 64)."""
    assert C % 64 == 0
    bf16 = mybir.dt.bfloat16
    f32 = mybir.dt.float32

    nc = bacc.Bacc("TRN2", target_bir_lowering=True, debug=False, num_devices=8)

    xT = nc.dram_tensor("xT", [128, KD, C], bf16, kind="ExternalInput")
    # Weights packed per-partition-contiguous:
    #   w1[p, f, k, j] = W1[k*128+p, f*128+j]   -> [128, KF*KD*128]
    #   w2[p, d, k2, j] = W2[k2*128+p, d*128+j] -> [128, KD*KF*128]
    w1 = nc.dram_tensor("w1", [128, KF * KD * 128], bf16, kind="ExternalInput")
    w2 = nc.dram_tensor("w2", [128, KD * KF * 128], bf16, kind="ExternalInput")
    b1 = nc.dram_tensor("b1", [128, KF], f32, kind="ExternalInput")
    b2 = nc.dram_tensor("b2", [128, KD], f32, kind="ExternalInput")
    yT = nc.dram_tensor("yT", [KD, 128, C], f32, kind="ExternalOutput")

    W1C = KD * 128          # SBUF cols per f-tile of w1
    W2C = KF * 128          # SBUF cols per d-tile of w2

    with tile.TileContext(nc) as tc:
        with (
            tc.tile_pool(name="const", bufs=1) as const,
            tc.tile_pool(name="xp", bufs=1) as xp,
            tc.tile_pool(name="hp", bufs=1) as hp,
            tc.tile_pool(name="yp", bufs=3) as yp,
            tc.tile_pool(name="psA", bufs=4, space="PSUM") as psA,
            tc.tile_pool(name="psB", bufs=3, space="PSUM") as psB,
            tc.tile_pool(name="psW", bufs=1, space="PSUM") as psW,
        ):
            w1_sb = const.tile([128, KF * W1C], bf16)
            w2_sb = const.tile([128, KD * W2C], bf16)
            b1_sb = const.tile([128, KF], f32)
            b2_sb = const.tile([128, KD], f32)
            warm = const.tile([128, NT], bf16)

            # One-time weight load, both HW DGE queues, paced so the head
            # of w1 lands first (the first matmuls need f=0 immediately).
            def w1_dma(q, f0, f1):
                q.dma_start(w1_sb[:, f0 * W1C : f1 * W1C], w1[:, f0 * W1C : f1 * W1C])

            def w2_dma(q, d0, d1):
                q.dma_start(w2_sb[:, d0 * W2C : d1 * W2C], w2[:, d0 * W2C : d1 * W2C])

            # Chunk 0's x tiles ride the scalar HWDGE queue: the SWDGE
            # (gpsimd) path has ~2-3us end-to-end latency per transfer,
            # which stalled the first chunk's k-loop long enough to
            # re-throttle the PE clock.  Later chunks prefetch a full
            # chunk ahead, so SWDGE latency is harmless there.
            chunks = _chunks(C)
            # Queue discipline: the scalar engine also runs the gelus, so it
            # gets ONLY the two PE-gating transfers (w1 f0 + chunk-0 x) and
            # is then free — a DMA-issue backlog there delays the first
            # gelu, fills psA, and stalls the PE.  Everything else rides the
            # sync queue in small-to-large granules paced to stay ahead of
            # the first chunk's consumption (one w1 f-tile per ~1.1us, one
            # w2 d-tile per ~4.4us).  y stores + later x prefetch take the
            # latency-tolerant SWDGE (gpsimd) queue.
            nc.gpsimd.memset(warm[:], 0.0)
            # Everything the first matmuls gate on rides the sync queue —
            # the scalar queue's first slot is the gelu table load (~1.3us),
            # which would delay these.
            w1_dma(nc.sync, 0, 1)
            x_t = xp.tile([128, KD, NT], bf16)
            w0 = chunks[0][1]
            nc.sync.dma_start(x_t[:, 0:3, :w0], xT[:, 0:3, 0:w0])
            nc.sync.dma_start(x_t[:, 3:6, :w0], xT[:, 3:6, 0:w0])
            nc.sync.dma_start(x_t[:, 6:KD, :w0], xT[:, 6:KD, 0:w0])
            x_prev = x_t
            nc.sync.dma_start(b1_sb[:], b1[:])
            nc.sync.dma_start(b2_sb[:], b2[:])
            w1_dma(nc.sync, 1, 2)
            w1_dma(nc.sync, 2, 3)
            w1_dma(nc.sync, 3, 4)
            for f in range(4, KF, 2):
                w1_dma(nc.sync, f, f + 2)
            for dd in range(KD):
                w2_dma(nc.sync, dd, dd + 1)

            # PE warmup: a few throwaway matmuls during the initial DMA wait
            # so the HAM clock gate is already at 8/8 when real work starts.
            psw = psW.tile([128, NT], f32)
            for _ in range(6):
                nc.tensor.matmul(psw[:], warm[:, :128], warm[:], start=True, stop=True)

            for ci, (c0, w) in enumerate(chunks):
                # xp has ONE buffer on purpose: chunk c+1's x DMA then
                # write-after-read waits on chunk c's mm1 — which both keeps
                # the SWDGE prefetch out of the kernel-head DMA window (the
                # scheduler hoists dep-free DMAs) and still lands a full
                # mm2-phase (~35us) before the data is needed.
                if ci == 0:
                    x_t = x_prev
                else:
                    x_t = xp.tile([128, KD, NT], bf16)
                    nc.gpsimd.dma_start(x_t[:, :, :w], xT[:, :, c0 : c0 + w])

                h_t = hp.tile([128, KF, NT], bf16)
                for f in range(KF):
                    ps = psA.tile([128, NT], f32)
                    for k in range(KD):
                        nc.tensor.matmul(
                            ps[:, :w],
                            w1_sb[:, (f * KD + k) * 128 : (f * KD + k + 1) * 128],
                            x_t[:, k, :w],
                            start=(k == 0),
                            stop=(k == KD - 1),
                        )
                    nc.scalar.activation(
                        h_t[:, f, :w],
                        ps[:, :w],
                        mybir.ActivationFunctionType.Gelu,
                        bias=b1_sb[:, f : f + 1],
                    )

                for d in range(KD):
                    ps2 = psB.tile([128, NT], f32)
                    for k2 in range(KF):
                        nc.tensor.matmul(
                            ps2[:, :w],
                            w2_sb[:, (d * KF + k2) * 128 : (d * KF + k2 + 1) * 128],
                            h_t[:, k2, :w],
                            start=(k2 == 0),
                            stop=(k2 == KF - 1),
                        )
                    y_t = yp.tile([128, NT], f32)
                    nc.vector.tensor_scalar_add(y_t[:, :w], ps2[:, :w], b2_sb[:, d : d + 1])
                    # last chunk's stores take the (by now idle) HWDGE sync
                    # queue: ~2us less completion latency on the kernel tail
                    yq = nc.sync if ci == len(chunks) - 1 else nc.gpsimd
                    yq.dma_start(yT[d, :, c0 : c0 + w], y_t[:, :w])

    nc.compile()
    return nc


def _get_kernel(C: int):
    if C not in _KERNEL_CACHE:
        _KERNEL_CACHE[C] = _build_kernel(C)
    return _KERNEL_CACHE[C]


def _route(xf, Wg, bg, top_k):
    """Replicate the reference gate: logits -> top-k -> softmax."""
    logits = xf.astype(np.float32) @ Wg.astype(np.float32) + bg.astype(np.float32)
    # jax.lax.top_k: values sorted descending, ties broken by lower index.
    order = np.argsort(-logits, axis=1, kind="stable")
    sel = order[:, :top_k]                                      # [T, K]
    vals = np.take_along_axis(logits, sel, axis=1)              # [T, K]
    vmax = vals.max(axis=1, keepdims=True)
    ex = np.exp((vals - vmax).astype(np.float32))
    w = ex / ex.sum(axis=1, keepdims=True)                      # [T, K]
    return sel, w.astype(np.float32)


def _plan(x, Wg, bg, top_k):
    """Routing plan: token indices + gate weight per expert, capacity C."""
    B, S, _ = x.shape
    xf = np.ascontiguousarray(x.reshape(B * S, D).astype(np.float32))
    sel, w = _route(xf, Wg, bg, top_k)
    idx_list, gate_list = [], []
    for e in range(E):
        hit = (sel == e)                    # [T, K]
        tok = np.nonzero(hit.any(axis=1))[0]
        kslot = hit[tok].argmax(axis=1)
        idx_list.append(tok)
        gate_list.append(w[tok, kslot])
    C = max(128, int(-(-max(len(t) for t in idx_list) // 64)) * 64)
    return xf, idx_list, gate_list, C


def _pack_inputs(xf, idx_list, C, W1, b1, W2, b2):
    xf_bf = xf.astype(ml_dtypes.bfloat16)
    in_maps = []
    for e in range(E):
        tok = idx_list[e]
        xe = np.zeros((C, D), dtype=ml_dtypes.bfloat16)
        xe[: len(tok)] = xf_bf[tok]
        in_maps.append(
            {
                "xT": np.ascontiguousarray(xe.reshape(C, KD, 128).transpose(2, 1, 0)),
                "w1": np.ascontiguousarray(
                    W1[e].astype(ml_dtypes.bfloat16)
                    .reshape(KD, 128, KF, 128).transpose(1, 2, 0, 3)
                    .reshape(128, KF * KD * 128)
                ),
                "w2": np.ascontiguousarray(
                    W2[e].astype(ml_dtypes.bfloat16)
                    .reshape(KF, 128, KD, 128).transpose(1, 2, 0, 3)
                    .reshape(128, KD * KF * 128)
                ),
                "b1": np.ascontiguousarray(b1[e].reshape(KF, 128).T.astype(np.float32)),
                "b2": np.ascontiguousarray(b2[e].reshape(KD, 128).T.astype(np.float32)),
            }
        )
    return in_maps


def _combine(results, idx_list, gate_list, C, T):
    out = np.zeros((T, D), dtype=np.float32)
    for e in range(E):
        tok = idx_list[e]
        if len(tok) == 0:
            continue
        y_pack = results[e]["yT"]                           # [KD, 128, C] f32
        ye = y_pack.transpose(2, 0, 1).reshape(C, D)[: len(tok)]
        out[tok] += gate_list[e][:, None] * ye
    return out


def kernel(x, W1, b1, W2, b2, Wg, bg, top_k):
    x = np.asarray(x)
    W1 = np.asarray(W1, dtype=np.float32)
    b1 = np.asarray(b1, dtype=np.float32)
    W2 = np.asarray(W2, dtype=np.float32)
    b2 = np.asarray(b2, dtype=np.float32)
    Wg = np.asarray(Wg, dtype=np.float32)
    bg = np.asarray(bg, dtype=np.float32)
    top_k = int(np.asarray(top_k))

    B, S, Din = x.shape
    xf, idx_list, gate_list, C = _plan(x, Wg, bg, top_k)
    nc = _get_kernel(C)
    in_maps = _pack_inputs(xf, idx_list, C, W1, b1, W2, b2)
    # The device pool occasionally throws a transient unrecoverable-exec
    # error on a fresh NEFF; a retry has always succeeded.
    last_err = None
    for _ in range(3):
        try:
            res = run_bass_kernel_spmd(nc, in_maps, list(range(E)))
            break
        except Exception as e:  # noqa: BLE001
            last_err = e
    else:
        raise last_err
    out = _combine(res.results, idx_list, gate_list, C, B * S)
    return out.reshape(B, S, Din).astype(np.float32)


# revision 27
# speedup vs baseline: 1.0047x; 1.0047x over previous
"""MoE layer (top-2 routing, 8 experts) on 8 Trainium2 NeuronCores.

Strategy (expert parallelism, per sharding hint):
  - Host computes the gate (logits -> top-k -> softmax) and routes tokens:
    expert e's tokens are gathered, padded to a common capacity C, and sent
    to core e.  This is the host-side equivalent of the "all-to-all tokens
    by expert assignment" step.
  - Core e runs the expert FFN for its tokens:
        yT = (gelu(x @ W1[e] + b1[e]) @ W2[e] + b2[e])^T
    entirely on-device in a transpose-free layout:
      mm1:  h^T[f,c] = sum_k W1_blk[k,f].T @ x^T[k,c]   (W1 stationary)
      gelu: ACT engine, exact (erf) Gelu, bias b1 fused
      mm2:  y^T[d,c] = sum_f W2_blk[f,d].T @ h^T[f,c]   (W2 stationary)
    Activations/weights are bf16 (full PE rate), accumulation fp32 in PSUM.
  - Both expert weight stacks (16.8 MB bf16) stay RESIDENT in SBUF: they are
    DMA'd exactly once, laid out per-partition-contiguous in DRAM so the
    load runs at full HBM bandwidth, split across the two hardware DGE
    queues (sync + scalar) and paced so arrival stays ahead of the first
    chunk's consumption.  Token chunks then stream through with zero
    steady-state weight traffic, keeping the PE at its bf16 roofline.
  - Host scatters y back, scaled by the gate weights, and sums the top-k
    expert contributions per token.

Hardcoded problem shape: x [4, 2048, 1024], E=8 experts, D=1024, F=4096.
"""

import numpy as np
import ml_dtypes

import concourse.bass as bass
import concourse.mybir as mybir
import concourse.tile as tile
from concourse import bacc
from concourse.bass_utils import run_bass_kernel_spmd

D = 1024
F = 4096
E = 8
KD = D // 128   # 8 k-tiles over D
KF = F // 128   # 32 k-tiles over F
NT = 512        # max token chunk width (one PSUM bank of fp32)

_KERNEL_CACHE = {}


def _chunks(C):
    """Token chunks: one full 512 first (covers the one-time weight load),
    then the rest in equal widths (multiples of 8, each <= 512).

    Chunks narrower than ~256 run LDWEIGHTS-bound on the PE (the stationary
    load no longer hides under the matmul), so avoid a tiny remainder chunk.
    """
    if C <= NT:
        return [(0, C)]
    first = 256
    k = -(-(C - first) // NT)
    base = ((C - first) // k) & ~7
    if base < 256:
        # fall back to plain equal split
        first = 0
        k = -(-C // NT)
        base = (C // k) & ~7
    rest = C - first
    rem = rest - base * k
    assert rem % 8 == 0
    widths = ([first] if first else []) + [base + 8 * (i < rem // 8) for i in range(k)]
    out, c0 = [], 0
    for w in widths:
        out.append((c0, w))
        c0 += w
    assert c0 == C
    return out


def _build_kernel(C: int):
    """Per-core expert-FFN kernel for capacity C (multiple of 64)."""
    assert C % 64 == 0
    bf16 = mybir.dt.bfloat16
    f32 = mybir.dt.float32

    nc = bacc.Bacc("TRN2", target_bir_lowering=True, debug=False, num_devices=8)

    xT = nc.dram_tensor("xT", [128, KD, C], bf16, kind="ExternalInput")
    # Weights packed per-partition-contiguous:
    #   w1[p, f, k, j] = W1[k*128+p, f*128+j]   -> [128, KF*KD*128]
    #   w2[p, d, k2, j] = W2[k2*128+p, d*128+j] -> [128, KD*KF*128]
    w1 = nc.dram_tensor("w1", [128, KF * KD * 128], bf16, kind="ExternalInput")
    w2 = nc.dram_tensor("w2", [128, KD * KF * 128], bf16, kind="ExternalInput")
    b1 = nc.dram_tensor("b1", [128, KF], f32, kind="ExternalInput")
    b2 = nc.dram_tensor("b2", [128, KD], f32, kind="ExternalInput")
    yT = nc.dram_tensor("yT", [KD, 128, C], f32, kind="ExternalOutput")

    W1C = KD * 128          # SBUF cols per f-tile of w1
    W2C = KF * 128          # SBUF cols per d-tile of w2

    with tile.TileContext(nc) as tc:
        with (
            tc.tile_pool(name="const", bufs=1) as const,
            tc.tile_pool(name="xp", bufs=1) as xp,
            tc.tile_pool(name="hp", bufs=1) as hp,
            tc.tile_pool(name="yp", bufs=3) as yp,
            tc.tile_pool(name="psA", bufs=4, space="PSUM") as psA,
            tc.tile_pool(name="psB", bufs=3, space="PSUM") as psB,
            tc.tile_pool(name="psW", bufs=1, space="PSUM") as psW,
        ):
            w1_sb = const.tile([128, KF * W1C], bf16)
            w2_sb = const.tile([128, KD * W2C], bf16)
            b1_sb = const.tile([128, KF], f32)
            b2_sb = const.tile([128, KD], f32)
            warm = const.tile([128, NT], bf16)

            # One-time weight load, both HW DGE queues, paced so the head
            # of w1 lands first (the first matmuls need f=0 immediately).
            def w1_dma(q, f0, f1):
                q.dma_start(w1_sb[:, f0 * W1C : f1 * W1C], w1[:, f0 * W1C : f1 * W1C])

            def w2_dma(q, d0, d1):
                q.dma_start(w2_sb[:, d0 * W2C : d1 * W2C], w2[:, d0 * W2C : d1 * W2C])

            # Chunk 0's x tiles ride the scalar HWDGE queue: the SWDGE
            # (gpsimd) path has ~2-3us end-to-end latency per transfer,
            # which stalled the first chunk's k-loop long enough to
            # re-throttle the PE clock.  Later chunks prefetch a full
            # chunk ahead, so SWDGE latency is harmless there.
            chunks = _chunks(C)
            # Queue discipline: the scalar engine also runs the gelus, so it
            # gets ONLY the two PE-gating transfers (w1 f0 + chunk-0 x) and
            # is then free — a DMA-issue backlog there delays the first
            # gelu, fills psA, and stalls the PE.  Everything else rides the
            # sync queue in small-to-large granules paced to stay ahead of
            # the first chunk's consumption (one w1 f-tile per ~1.1us, one
            # w2 d-tile per ~4.4us).  y stores + later x prefetch take the
            # latency-tolerant SWDGE (gpsimd) queue.
            nc.gpsimd.memset(warm[:], 0.0)
            w1_dma(nc.sync, 0, 1)
            x_t = xp.tile([128, KD, NT], bf16)
            w0 = chunks[0][1]
            nc.scalar.dma_start(x_t[:, 0:4, :w0], xT[:, 0:4, 0:w0])
            nc.scalar.dma_start(x_t[:, 4:KD, :w0], xT[:, 4:KD, 0:w0])
            x_prev = x_t
            nc.sync.dma_start(b1_sb[:], b1[:])
            nc.sync.dma_start(b2_sb[:], b2[:])
            w1_dma(nc.sync, 1, 2)
            w1_dma(nc.sync, 2, 3)
            w1_dma(nc.sync, 3, 4)
            for f in range(4, KF, 2):
                w1_dma(nc.sync, f, f + 2)
            for dd in range(KD):
                w2_dma(nc.sync, dd, dd + 1)

            # PE warmup: a few throwaway matmuls during the initial DMA wait
            # so the HAM clock gate is already at 8/8 when real work starts.
            psw = psW.tile([128, NT], f32)
            for _ in range(10):
                nc.tensor.matmul(psw[:], warm[:, :128], warm[:], start=True, stop=True)

            for ci, (c0, w) in enumerate(chunks):
                # xp has ONE buffer on purpose: chunk c+1's x DMA then
                # write-after-read waits on chunk c's mm1 — which both keeps
                # the SWDGE prefetch out of the kernel-head DMA window (the
                # scheduler hoists dep-free DMAs) and still lands a full
                # mm2-phase (~35us) before the data is needed.
                if ci == 0:
                    x_t = x_prev
                else:
                    x_t = xp.tile([128, KD, NT], bf16)
                    nc.gpsimd.dma_start(x_t[:, :, :w], xT[:, :, c0 : c0 + w])

                h_t = hp.tile([128, KF, NT], bf16)
                for f in range(KF):
                    ps = psA.tile([128, NT], f32)
                    for k in range(KD):
                        nc.tensor.matmul(
                            ps[:, :w],
                            w1_sb[:, (f * KD + k) * 128 : (f * KD + k + 1) * 128],
                            x_t[:, k, :w],
                            start=(k == 0),
                            stop=(k == KD - 1),
                        )
                    nc.scalar.activation(
                        h_t[:, f, :w],
                        ps[:, :w],
                        mybir.ActivationFunctionType.Gelu,
                        bias=b1_sb[:, f : f + 1],
                    )

                for d in range(KD):
                    ps2 = psB.tile([128, NT], f32)
                    for k2 in range(KF):
                        nc.tensor.matmul(
                            ps2[:, :w],
                            w2_sb[:, (d * KF + k2) * 128 : (d * KF + k2 + 1) * 128],
                            h_t[:, k2, :w],
                            start=(k2 == 0),
                            stop=(k2 == KF - 1),
                        )
                    y_t = yp.tile([128, NT], f32)
                    nc.vector.tensor_scalar_add(y_t[:, :w], ps2[:, :w], b2_sb[:, d : d + 1])
                    # last chunk's stores take the (by now idle) HWDGE sync
                    # queue: ~2us less completion latency on the kernel tail
                    yq = nc.sync if ci == len(chunks) - 1 else nc.gpsimd
                    yq.dma_start(yT[d, :, c0 : c0 + w], y_t[:, :w])

    nc.compile()
    return nc


def _get_kernel(C: int):
    if C not in _KERNEL_CACHE:
        _KERNEL_CACHE[C] = _build_kernel(C)
    return _KERNEL_CACHE[C]


def _route(xf, Wg, bg, top_k):
    """Replicate the reference gate: logits -> top-k -> softmax."""
    logits = xf.astype(np.float32) @ Wg.astype(np.float32) + bg.astype(np.float32)
    # jax.lax.top_k: values sorted descending, ties broken by lower index.
    order = np.argsort(-logits, axis=1, kind="stable")
    sel = order[:, :top_k]                                      # [T, K]
    vals = np.take_along_axis(logits, sel, axis=1)              # [T, K]
    vmax = vals.max(axis=1, keepdims=True)
    ex = np.exp((vals - vmax).astype(np.float32))
    w = ex / ex.sum(axis=1, keepdims=True)                      # [T, K]
    return sel, w.astype(np.float32)


def _plan(x, Wg, bg, top_k):
    """Routing plan: token indices + gate weight per expert, capacity C."""
    B, S, _ = x.shape
    xf = np.ascontiguousarray(x.reshape(B * S, D).astype(np.float32))
    sel, w = _route(xf, Wg, bg, top_k)
    idx_list, gate_list = [], []
    for e in range(E):
        hit = (sel == e)                    # [T, K]
        tok = np.nonzero(hit.any(axis=1))[0]
        kslot = hit[tok].argmax(axis=1)
        idx_list.append(tok)
        gate_list.append(w[tok, kslot])
    C = max(128, int(-(-max(len(t) for t in idx_list) // 64)) * 64)
    return xf, idx_list, gate_list, C


def _pack_inputs(xf, idx_list, C, W1, b1, W2, b2):
    xf_bf = xf.astype(ml_dtypes.bfloat16)
    in_maps = []
    for e in range(E):
        tok = idx_list[e]
        xe = np.zeros((C, D), dtype=ml_dtypes.bfloat16)
        xe[: len(tok)] = xf_bf[tok]
        in_maps.append(
            {
                "xT": np.ascontiguousarray(xe.reshape(C, KD, 128).transpose(2, 1, 0)),
                "w1": np.ascontiguousarray(
                    W1[e].astype(ml_dtypes.bfloat16)
                    .reshape(KD, 128, KF, 128).transpose(1, 2, 0, 3)
                    .reshape(128, KF * KD * 128)
                ),
                "w2": np.ascontiguousarray(
                    W2[e].astype(ml_dtypes.bfloat16)
                    .reshape(KF, 128, KD, 128).transpose(1, 2, 0, 3)
                    .reshape(128, KD * KF * 128)
                ),
                "b1": np.ascontiguousarray(b1[e].reshape(KF, 128).T.astype(np.float32)),
                "b2": np.ascontiguousarray(b2[e].reshape(KD, 128).T.astype(np.float32)),
            }
        )
    return in_maps


def _combine(results, idx_list, gate_list, C, T):
    out = np.zeros((T, D), dtype=np.float32)
    for e in range(E):
        tok = idx_list[e]
        if len(tok) == 0:
            continue
        y_pack = results[e]["yT"]                           # [KD, 128, C] f32
        ye = y_pack.transpose(2, 0, 1).reshape(C, D)[: len(tok)]
        out[tok] += gate_list[e][:, None] * ye
    return out


def kernel(x, W1, b1, W2, b2, Wg, bg, top_k):
    x = np.asarray(x)
    W1 = np.asarray(W1, dtype=np.float32)
    b1 = np.asarray(b1, dtype=np.float32)
    W2 = np.asarray(W2, dtype=np.float32)
    b2 = np.asarray(b2, dtype=np.float32)
    Wg = np.asarray(Wg, dtype=np.float32)
    bg = np.asarray(bg, dtype=np.float32)
    top_k = int(np.asarray(top_k))

    B, S, Din = x.shape
    xf, idx_list, gate_list, C = _plan(x, Wg, bg, top_k)
    nc = _get_kernel(C)
    in_maps = _pack_inputs(xf, idx_list, C, W1, b1, W2, b2)
    # The device pool occasionally throws a transient unrecoverable-exec
    # error on a fresh NEFF; a retry has always succeeded.
    last_err = None
    for _ in range(3):
        try:
            res = run_bass_kernel_spmd(nc, in_maps, list(range(E)))
            break
        except Exception as e:  # noqa: BLE001
            last_err = e
    else:
        raise last_err
    out = _combine(res.results, idx_list, gate_list, C, B * S)
    return out.reshape(B, S, Din).astype(np.float32)


# revision 29
# speedup vs baseline: 1.0053x; 1.0005x over previous
"""MoE layer (top-2 routing, 8 experts) on 8 Trainium2 NeuronCores.

Strategy (expert parallelism, per sharding hint):
  - Host computes the gate (logits -> top-k -> softmax) and routes tokens:
    expert e's tokens are gathered, padded to a common capacity C, and sent
    to core e.  This is the host-side equivalent of the "all-to-all tokens
    by expert assignment" step.
  - Core e runs the expert FFN for its tokens:
        yT = (gelu(x @ W1[e] + b1[e]) @ W2[e] + b2[e])^T
    entirely on-device in a transpose-free layout:
      mm1:  h^T[f,c] = sum_k W1_blk[k,f].T @ x^T[k,c]   (W1 stationary)
      gelu: ACT engine, exact (erf) Gelu, bias b1 fused
      mm2:  y^T[d,c] = sum_f W2_blk[f,d].T @ h^T[f,c]   (W2 stationary)
    Activations/weights are bf16 (full PE rate), accumulation fp32 in PSUM.
  - Both expert weight stacks (16.8 MB bf16) stay RESIDENT in SBUF: they are
    DMA'd exactly once, laid out per-partition-contiguous in DRAM so the
    load runs at full HBM bandwidth, split across the two hardware DGE
    queues (sync + scalar) and paced so arrival stays ahead of the first
    chunk's consumption.  Token chunks then stream through with zero
    steady-state weight traffic, keeping the PE at its bf16 roofline.
  - Host scatters y back, scaled by the gate weights, and sums the top-k
    expert contributions per token.

Schedule details (why this hits ~476us vs the 457us pure-matmul floor of
capacity*512 moving columns at 2.4GHz + 2.5ns/matmul dispatch):
  - token chunks [256, ~464 x 4]: chunks below ~256 go LDWEIGHTS-bound;
    the leading 512-col-equivalent of work covers the one-time weight load.
  - w1 arrives as singles-then-pairs on sync, always >~1us ahead of the
    f-loop's consumption; w2 follows on the same queue ahead of mm2.
  - chunk-0 x rides the scalar HWDGE queue (2 transfers); later chunks use
    one fused SWDGE transfer, write-after-read gated by the single x buffer
    so the scheduler cannot hoist it into the cold-fabric kernel head.
  - 10 warm-up matmuls on zeroed SBUF bring the PE's HAM clock gate to 8/8
    during the initial DMA wait.
  - y stores ride SWDGE except the last chunk (sync: ~2us less completion
    latency ahead of the exit barrier).

Hardcoded problem shape: x [4, 2048, 1024], E=8 experts, D=1024, F=4096.
"""

import numpy as np
import ml_dtypes

import concourse.bass as bass
import concourse.mybir as mybir
import concourse.tile as tile
from concourse import bacc
from concourse.bass_utils import run_bass_kernel_spmd

D = 1024
F = 4096
E = 8
KD = D // 128   # 8 k-tiles over D
KF = F // 128   # 32 k-tiles over F
NT = 512        # max token chunk width (one PSUM bank of fp32)

_KERNEL_CACHE = {}


def _chunks(C):
    """Token chunks: one full 512 first (covers the one-time weight load),
    then the rest in equal widths (multiples of 8, each <= 512).

    Chunks narrower than ~256 run LDWEIGHTS-bound on the PE (the stationary
    load no longer hides under the matmul), so avoid a tiny remainder chunk.
    """
    if C <= NT:
        return [(0, C)]
    first = 256
    k = -(-(C - first) // NT)
    base = ((C - first) // k) & ~7
    if base < 256:
        # fall back to plain equal split
        first = 0
        k = -(-C // NT)
        base = (C // k) & ~7
    rest = C - first
    rem = rest - base * k
    assert rem % 8 == 0
    widths = ([first] if first else []) + [base + 8 * (i < rem // 8) for i in range(k)]
    out, c0 = [], 0
    for w in widths:
        out.append((c0, w))
        c0 += w
    assert c0 == C
    return out


def _build_kernel(C: int):
    """Per-core expert-FFN kernel for capacity C (multiple of 64)."""
    assert C % 64 == 0
    bf16 = mybir.dt.bfloat16
    f32 = mybir.dt.float32

    nc = bacc.Bacc("TRN2", target_bir_lowering=True, debug=False, num_devices=8)

    xT = nc.dram_tensor("xT", [128, KD, C], bf16, kind="ExternalInput")
    # Weights packed per-partition-contiguous:
    #   w1[p, f, k, j] = W1[k*128+p, f*128+j]   -> [128, KF*KD*128]
    #   w2[p, d, k2, j] = W2[k2*128+p, d*128+j] -> [128, KD*KF*128]
    w1 = nc.dram_tensor("w1", [128, KF * KD * 128], bf16, kind="ExternalInput")
    w2 = nc.dram_tensor("w2", [128, KD * KF * 128], bf16, kind="ExternalInput")
    b1 = nc.dram_tensor("b1", [128, KF], f32, kind="ExternalInput")
    b2 = nc.dram_tensor("b2", [128, KD], f32, kind="ExternalInput")
    yT = nc.dram_tensor("yT", [KD, 128, C], f32, kind="ExternalOutput")

    W1C = KD * 128          # SBUF cols per f-tile of w1
    W2C = KF * 128          # SBUF cols per d-tile of w2

    with tile.TileContext(nc) as tc:
        with (
            tc.tile_pool(name="const", bufs=1) as const,
            tc.tile_pool(name="xp", bufs=1) as xp,
            tc.tile_pool(name="hp", bufs=1) as hp,
            tc.tile_pool(name="yp", bufs=3) as yp,
            tc.tile_pool(name="psA", bufs=4, space="PSUM") as psA,
            tc.tile_pool(name="psB", bufs=3, space="PSUM") as psB,
            tc.tile_pool(name="psW", bufs=1, space="PSUM") as psW,
        ):
            w1_sb = const.tile([128, KF * W1C], bf16)
            w2_sb = const.tile([128, KD * W2C], bf16)
            b1_sb = const.tile([128, KF], f32)
            b2_sb = const.tile([128, KD], f32)
            warm = const.tile([128, NT], bf16)

            # One-time weight load, both HW DGE queues, paced so the head
            # of w1 lands first (the first matmuls need f=0 immediately).
            def w1_dma(q, f0, f1):
                q.dma_start(w1_sb[:, f0 * W1C : f1 * W1C], w1[:, f0 * W1C : f1 * W1C])

            def w2_dma(q, d0, d1):
                q.dma_start(w2_sb[:, d0 * W2C : d1 * W2C], w2[:, d0 * W2C : d1 * W2C])

            # Chunk 0's x tiles ride the scalar HWDGE queue: the SWDGE
            # (gpsimd) path has ~2-3us end-to-end latency per transfer,
            # which stalled the first chunk's k-loop long enough to
            # re-throttle the PE clock.  Later chunks prefetch a full
            # chunk ahead, so SWDGE latency is harmless there.
            chunks = _chunks(C)
            # Queue discipline: the scalar engine also runs the gelus, so it
            # gets ONLY the two PE-gating transfers (w1 f0 + chunk-0 x) and
            # is then free — a DMA-issue backlog there delays the first
            # gelu, fills psA, and stalls the PE.  Everything else rides the
            # sync queue in small-to-large granules paced to stay ahead of
            # the first chunk's consumption (one w1 f-tile per ~1.1us, one
            # w2 d-tile per ~4.4us).  y stores + later x prefetch take the
            # latency-tolerant SWDGE (gpsimd) queue.
            nc.gpsimd.memset(warm[:], 0.0)
            w1_dma(nc.sync, 0, 1)
            x_t = xp.tile([128, KD, NT], bf16)
            w0 = chunks[0][1]
            nc.scalar.dma_start(x_t[:, 0:4, :w0], xT[:, 0:4, 0:w0])
            nc.scalar.dma_start(x_t[:, 4:KD, :w0], xT[:, 4:KD, 0:w0])
            x_prev = x_t
            nc.sync.dma_start(b1_sb[:], b1[:])
            nc.sync.dma_start(b2_sb[:], b2[:])
            w1_dma(nc.sync, 1, 2)
            w1_dma(nc.sync, 2, 3)
            w1_dma(nc.sync, 3, 4)
            for f in range(4, KF, 2):
                w1_dma(nc.sync, f, f + 2)
            for dd in range(KD):
                w2_dma(nc.sync, dd, dd + 1)

            # PE warmup: a few throwaway matmuls during the initial DMA wait
            # so the HAM clock gate is already at 8/8 when real work starts.
            psw = psW.tile([128, NT], f32)
            for _ in range(10):
                nc.tensor.matmul(psw[:], warm[:, :128], warm[:], start=True, stop=True)

            for ci, (c0, w) in enumerate(chunks):
                # xp has ONE buffer on purpose: chunk c+1's x DMA then
                # write-after-read waits on chunk c's mm1 — which both keeps
                # the SWDGE prefetch out of the kernel-head DMA window (the
                # scheduler hoists dep-free DMAs) and still lands a full
                # mm2-phase (~35us) before the data is needed.
                if ci == 0:
                    x_t = x_prev
                else:
                    x_t = xp.tile([128, KD, NT], bf16)
                    nc.gpsimd.dma_start(x_t[:, :, :w], xT[:, :, c0 : c0 + w])

                h_t = hp.tile([128, KF, NT], bf16)
                for f in range(KF):
                    ps = psA.tile([128, NT], f32)
                    for k in range(KD):
                        nc.tensor.matmul(
                            ps[:, :w],
                            w1_sb[:, (f * KD + k) * 128 : (f * KD + k + 1) * 128],
                            x_t[:, k, :w],
                            start=(k == 0),
                            stop=(k == KD - 1),
                        )
                    nc.scalar.activation(
                        h_t[:, f, :w],
                        ps[:, :w],
                        mybir.ActivationFunctionType.Gelu,
                        bias=b1_sb[:, f : f + 1],
                    )

                for d in range(KD):
                    ps2 = psB.tile([128, NT], f32)
                    for k2 in range(KF):
                        nc.tensor.matmul(
                            ps2[:, :w],
                            w2_sb[:, (d * KF + k2) * 128 : (d * KF + k2 + 1) * 128],
                            h_t[:, k2, :w],
                            start=(k2 == 0),
                            stop=(k2 == KF - 1),
                        )
                    y_t = yp.tile([128, NT], f32)
                    nc.vector.tensor_scalar_add(y_t[:, :w], ps2[:, :w], b2_sb[:, d : d + 1])
                    # last chunk's stores take the (by now idle) HWDGE sync
                    # queue: ~2us less completion latency on the kernel tail
                    yq = nc.sync if ci == len(chunks) - 1 else nc.gpsimd
                    yq.dma_start(yT[d, :, c0 : c0 + w], y_t[:, :w])

    nc.compile()
    return nc


def _get_kernel(C: int):
    if C not in _KERNEL_CACHE:
        _KERNEL_CACHE[C] = _build_kernel(C)
    return _KERNEL_CACHE[C]


def _route(xf, Wg, bg, top_k):
    """Replicate the reference gate: logits -> top-k -> softmax."""
    logits = xf.astype(np.float32) @ Wg.astype(np.float32) + bg.astype(np.float32)
    # jax.lax.top_k: values sorted descending, ties broken by lower index.
    order = np.argsort(-logits, axis=1, kind="stable")
    sel = order[:, :top_k]                                      # [T, K]
    vals = np.take_along_axis(logits, sel, axis=1)              # [T, K]
    vmax = vals.max(axis=1, keepdims=True)
    ex = np.exp((vals - vmax).astype(np.float32))
    w = ex / ex.sum(axis=1, keepdims=True)                      # [T, K]
    return sel, w.astype(np.float32)


def _plan(x, Wg, bg, top_k):
    """Routing plan: token indices + gate weight per expert, capacity C."""
    B, S, _ = x.shape
    xf = np.ascontiguousarray(x.reshape(B * S, D).astype(np.float32))
    sel, w = _route(xf, Wg, bg, top_k)
    idx_list, gate_list = [], []
    for e in range(E):
        hit = (sel == e)                    # [T, K]
        tok = np.nonzero(hit.any(axis=1))[0]
        kslot = hit[tok].argmax(axis=1)
        idx_list.append(tok)
        gate_list.append(w[tok, kslot])
    C = max(128, int(-(-max(len(t) for t in idx_list) // 64)) * 64)
    return xf, idx_list, gate_list, C


def _pack_inputs(xf, idx_list, C, W1, b1, W2, b2):
    xf_bf = xf.astype(ml_dtypes.bfloat16)
    in_maps = []
    for e in range(E):
        tok = idx_list[e]
        xe = np.zeros((C, D), dtype=ml_dtypes.bfloat16)
        xe[: len(tok)] = xf_bf[tok]
        in_maps.append(
            {
                "xT": np.ascontiguousarray(xe.reshape(C, KD, 128).transpose(2, 1, 0)),
                "w1": np.ascontiguousarray(
                    W1[e].astype(ml_dtypes.bfloat16)
                    .reshape(KD, 128, KF, 128).transpose(1, 2, 0, 3)
                    .reshape(128, KF * KD * 128)
                ),
                "w2": np.ascontiguousarray(
                    W2[e].astype(ml_dtypes.bfloat16)
                    .reshape(KF, 128, KD, 128).transpose(1, 2, 0, 3)
                    .reshape(128, KD * KF * 128)
                ),
                "b1": np.ascontiguousarray(b1[e].reshape(KF, 128).T.astype(np.float32)),
                "b2": np.ascontiguousarray(b2[e].reshape(KD, 128).T.astype(np.float32)),
            }
        )
    return in_maps


def _combine(results, idx_list, gate_list, C, T):
    out = np.zeros((T, D), dtype=np.float32)
    for e in range(E):
        tok = idx_list[e]
        if len(tok) == 0:
            continue
        y_pack = results[e]["yT"]                           # [KD, 128, C] f32
        ye = y_pack.transpose(2, 0, 1).reshape(C, D)[: len(tok)]
        out[tok] += gate_list[e][:, None] * ye
    return out


def kernel(x, W1, b1, W2, b2, Wg, bg, top_k):
    x = np.asarray(x)
    W1 = np.asarray(W1, dtype=np.float32)
    b1 = np.asarray(b1, dtype=np.float32)
    W2 = np.asarray(W2, dtype=np.float32)
    b2 = np.asarray(b2, dtype=np.float32)
    Wg = np.asarray(Wg, dtype=np.float32)
    bg = np.asarray(bg, dtype=np.float32)
    top_k = int(np.asarray(top_k))

    B, S, Din = x.shape
    xf, idx_list, gate_list, C = _plan(x, Wg, bg, top_k)
    in_maps = _pack_inputs(xf, idx_list, C, W1, b1, W2, b2)
    results = _run_device(C, in_maps)
    out = _combine(results, idx_list, gate_list, C, B * S)
    return out.reshape(B, S, Din).astype(np.float32)


def _run_device(C, in_maps):
    """Execute on the 8 cores, surviving the pool's transient device wedge.

    ~1 in 3 fresh sessions throws NRT_EXEC_UNIT_UNRECOVERABLE at NEFF load
    and the whole in-process jax/axon session stays poisoned, so after one
    in-process retry we re-run in a fresh subprocess (fresh device claim),
    which has always succeeded.
    """
    last_err = None
    for _ in range(2):
        try:
            return run_bass_kernel_spmd(_get_kernel(C), in_maps, list(range(E))).results
        except Exception as e:  # noqa: BLE001
            last_err = e
    import subprocess
    import sys
    import tempfile
    import os

    for _ in range(2):
        tmpd = tempfile.mkdtemp(prefix="moe_kernel_")
        in_path = os.path.join(tmpd, "in.npz")
        out_path = os.path.join(tmpd, "out.npz")
        payload = {"C": np.int64(C)}
        for i, m in enumerate(in_maps):
            for k, v in m.items():
                payload[f"{i}|{k}"] = (
                    v.view(np.uint16) if v.dtype == ml_dtypes.bfloat16 else v
                )
        np.savez(in_path, **payload)
        proc = subprocess.run(
            [sys.executable, os.path.abspath(__file__), in_path, out_path],
            capture_output=True,
            text=True,
        )
        if proc.returncode == 0 and os.path.exists(out_path):
            with np.load(out_path) as z:
                return [{"yT": z[f"{i}|yT"]} for i in range(E)]
        last_err = RuntimeError(
            f"subprocess retry failed (rc={proc.returncode}):\n{proc.stderr[-2000:]}"
        )
    raise last_err


def _subprocess_main(in_path, out_path):
    with np.load(in_path) as z:
        C = int(z["C"])
        in_maps = [{} for _ in range(E)]
        for key in z.files:
            if key == "C":
                continue
            i, name = key.split("|")
            v = z[key]
            if v.dtype == np.uint16:
                v = v.view(ml_dtypes.bfloat16)
            in_maps[int(i)][name] = v
    res = run_bass_kernel_spmd(_get_kernel(C), in_maps, list(range(E)))
    np.savez(out_path, **{f"{i}|yT": r["yT"] for i, r in enumerate(res.results)})


if __name__ == "__main__":
    import sys as _sys

    _subprocess_main(_sys.argv[1], _sys.argv[2])


# revision 32
# speedup vs baseline: 1.0085x; 1.0032x over previous
"""MoE layer (top-2 routing, 8 experts) on 8 Trainium2 NeuronCores.

Strategy (expert parallelism, per sharding hint):
  - Host computes the gate (logits -> top-k -> softmax) and routes tokens:
    expert e's tokens are gathered, padded to a common capacity C, and sent
    to core e.  This is the host-side equivalent of the "all-to-all tokens
    by expert assignment" step.
  - Core e runs the expert FFN for its tokens:
        yT = (gelu(x @ W1[e] + b1[e]) @ W2[e] + b2[e])^T
    entirely on-device in a transpose-free layout:
      mm1:  h^T[f,c] = sum_k W1_blk[k,f].T @ x^T[k,c]   (W1 stationary)
      gelu: ACT engine, exact (erf) Gelu, bias b1 fused
      mm2:  y^T[d,c] = sum_f W2_blk[f,d].T @ h^T[f,c]   (W2 stationary)
    Activations/weights are bf16 (full PE rate), accumulation fp32 in PSUM.
  - Both expert weight stacks (16.8 MB bf16) stay RESIDENT in SBUF: they are
    DMA'd exactly once, laid out per-partition-contiguous in DRAM so the
    load runs at full HBM bandwidth, split across the two hardware DGE
    queues (sync + scalar) and paced so arrival stays ahead of the first
    chunk's consumption.  Token chunks then stream through with zero
    steady-state weight traffic, keeping the PE at its bf16 roofline.
  - Host scatters y back, scaled by the gate weights, and sums the top-k
    expert contributions per token.

Schedule details (why this hits ~476us vs the 457us pure-matmul floor of
capacity*512 moving columns at 2.4GHz + 2.5ns/matmul dispatch):
  - token chunks [256, ~464 x 4]: chunks below ~256 go LDWEIGHTS-bound;
    the leading 512-col-equivalent of work covers the one-time weight load.
  - w1 arrives as singles-then-pairs on sync, always >~1us ahead of the
    f-loop's consumption; w2 follows on the same queue ahead of mm2.
  - chunk-0 x rides the scalar HWDGE queue (2 transfers); later chunks use
    one fused SWDGE transfer, write-after-read gated by the single x buffer
    so the scheduler cannot hoist it into the cold-fabric kernel head.
  - 10 warm-up matmuls on zeroed SBUF bring the PE's HAM clock gate to 8/8
    during the initial DMA wait.
  - y stores ride SWDGE except the last chunk (sync: ~2us less completion
    latency ahead of the exit barrier).

Hardcoded problem shape: x [4, 2048, 1024], E=8 experts, D=1024, F=4096.
"""

import numpy as np
import ml_dtypes

import concourse.bass as bass
import concourse.mybir as mybir
import concourse.tile as tile
from concourse import bacc
from concourse.bass_utils import run_bass_kernel_spmd

D = 1024
F = 4096
E = 8
KD = D // 128   # 8 k-tiles over D
KF = F // 128   # 32 k-tiles over F
NT = 512        # max token chunk width (one PSUM bank of fp32)

_KERNEL_CACHE = {}


def _chunks(C):
    """Token chunks: a short (256) head chunk, then equal widths (multiples
    of 8, each <= 512).

    The head chunk keeps the kernel-head x transfer small; chunks narrower
    than ~256 would run LDWEIGHTS-bound on the PE (the stationary load no
    longer hides under the matmul), so avoid a tiny remainder chunk.
    """
    if C <= NT:
        return [(0, C)]
    first = 256
    k = -(-(C - first) // NT)
    base = ((C - first) // k) & ~7
    if base < 256:
        # fall back to plain equal split
        first = 0
        k = -(-C // NT)
        base = (C // k) & ~7
    rest = C - first
    rem = rest - base * k
    assert rem % 8 == 0
    widths = ([first] if first else []) + [base + 8 * (i < rem // 8) for i in range(k)]
    out, c0 = [], 0
    for w in widths:
        out.append((c0, w))
        c0 += w
    assert c0 == C
    return out


def _build_kernel(C: int):
    """Per-core expert-FFN kernel for capacity C (multiple of 64)."""
    assert C % 64 == 0
    bf16 = mybir.dt.bfloat16
    f32 = mybir.dt.float32

    nc = bacc.Bacc("TRN2", target_bir_lowering=False, debug=False, num_devices=8)

    xT = nc.dram_tensor("xT", [128, KD, C], bf16, kind="ExternalInput")
    # Weights packed per-partition-contiguous:
    #   w1[p, f, k, j] = W1[k*128+p, f*128+j]   -> [128, KF*KD*128]
    #   w2[p, d, k2, j] = W2[k2*128+p, d*128+j] -> [128, KD*KF*128]
    w1 = nc.dram_tensor("w1", [128, KF * KD * 128], bf16, kind="ExternalInput")
    w2 = nc.dram_tensor("w2", [128, KD * KF * 128], bf16, kind="ExternalInput")
    b1 = nc.dram_tensor("b1", [128, KF], f32, kind="ExternalInput")
    b2 = nc.dram_tensor("b2", [128, KD], f32, kind="ExternalInput")
    yT = nc.dram_tensor("yT", [KD, 128, C], f32, kind="ExternalOutput")

    W1C = KD * 128          # SBUF cols per f-tile of w1
    W2C = KF * 128          # SBUF cols per d-tile of w2

    with tile.TileContext(nc) as tc:
        with (
            tc.tile_pool(name="const", bufs=1) as const,
            tc.tile_pool(name="xp", bufs=1) as xp,
            tc.tile_pool(name="hp", bufs=1) as hp,
            tc.tile_pool(name="yp", bufs=3) as yp,
            tc.tile_pool(name="psA", bufs=4, space="PSUM") as psA,
            tc.tile_pool(name="psB", bufs=3, space="PSUM") as psB,
            tc.tile_pool(name="psW", bufs=1, space="PSUM") as psW,
        ):
            w1_sb = const.tile([128, KF * W1C], bf16)
            w2_sb = const.tile([128, KD * W2C], bf16)
            b1_sb = const.tile([128, KF], f32)
            b2_sb = const.tile([128, KD], f32)
            warm = const.tile([128, NT], bf16)

            # One-time weight load, both HW DGE queues, paced so the head
            # of w1 lands first (the first matmuls need f=0 immediately).
            def w1_dma(q, f0, f1):
                q.dma_start(w1_sb[:, f0 * W1C : f1 * W1C], w1[:, f0 * W1C : f1 * W1C])

            def w2_dma(q, d0, d1):
                q.dma_start(w2_sb[:, d0 * W2C : d1 * W2C], w2[:, d0 * W2C : d1 * W2C])

            # Queue discipline: the scalar engine also runs the gelus, so it
            # gets ONLY the PE-gating chunk-0 x transfers and is then free —
            # a DMA-issue backlog there delays the first gelu, fills psA,
            # and stalls the PE.  Everything else rides the sync queue in
            # small-to-large granules paced to stay ahead of the first
            # chunk's consumption (one w1 f-tile per ~0.9us, one w2 d-tile
            # per ~3.6us).  y stores + later x prefetch take the
            # latency-tolerant SWDGE (gpsimd) queue.
            chunks = _chunks(C)
            nc.gpsimd.memset(warm[:], 0.0)
            w1_dma(nc.sync, 0, 1)
            x_t = xp.tile([128, KD, NT], bf16)
            w0 = chunks[0][1]
            nc.scalar.dma_start(x_t[:, 0:4, :w0], xT[:, 0:4, 0:w0])
            nc.scalar.dma_start(x_t[:, 4:KD, :w0], xT[:, 4:KD, 0:w0])
            x_prev = x_t
            nc.sync.dma_start(b1_sb[:], b1[:])
            nc.sync.dma_start(b2_sb[:], b2[:])
            w1_dma(nc.sync, 1, 2)
            w1_dma(nc.sync, 2, 3)
            w1_dma(nc.sync, 3, 4)
            for f in range(4, KF, 2):
                w1_dma(nc.sync, f, f + 2)
            for dd in range(KD):
                w2_dma(nc.sync, dd, dd + 1)

            # PE warmup: a few throwaway matmuls during the initial DMA wait
            # so the HAM clock gate is already at 8/8 when real work starts.
            psw = psW.tile([128, NT], f32)
            for _ in range(10):
                nc.tensor.matmul(psw[:], warm[:, :128], warm[:], start=True, stop=True)

            for ci, (c0, w) in enumerate(chunks):
                # xp has ONE buffer on purpose: chunk c+1's x DMA then
                # write-after-read waits on chunk c's mm1 — which both keeps
                # the SWDGE prefetch out of the kernel-head DMA window (the
                # scheduler hoists dep-free DMAs) and still lands a full
                # mm2-phase (~35us) before the data is needed.
                if ci == 0:
                    x_t = x_prev
                else:
                    x_t = xp.tile([128, KD, NT], bf16)
                    nc.gpsimd.dma_start(x_t[:, :, :w], xT[:, :, c0 : c0 + w])

                h_t = hp.tile([128, KF, NT], bf16)
                for f in range(KF):
                    ps = psA.tile([128, NT], f32)
                    for k in range(KD):
                        nc.tensor.matmul(
                            ps[:, :w],
                            w1_sb[:, (f * KD + k) * 128 : (f * KD + k + 1) * 128],
                            x_t[:, k, :w],
                            start=(k == 0),
                            stop=(k == KD - 1),
                        )
                    nc.scalar.activation(
                        h_t[:, f, :w],
                        ps[:, :w],
                        mybir.ActivationFunctionType.Gelu,
                        bias=b1_sb[:, f : f + 1],
                    )

                for d in range(KD):
                    ps2 = psB.tile([128, NT], f32)
                    for k2 in range(KF):
                        nc.tensor.matmul(
                            ps2[:, :w],
                            w2_sb[:, (d * KF + k2) * 128 : (d * KF + k2 + 1) * 128],
                            h_t[:, k2, :w],
                            start=(k2 == 0),
                            stop=(k2 == KF - 1),
                        )
                    y_t = yp.tile([128, NT], f32)
                    nc.vector.tensor_scalar_add(y_t[:, :w], ps2[:, :w], b2_sb[:, d : d + 1])
                    # last chunk's stores take the (by now idle) HWDGE sync
                    # queue: ~2us less completion latency on the kernel tail
                    yq = nc.sync if ci == len(chunks) - 1 else nc.gpsimd
                    yq.dma_start(yT[d, :, c0 : c0 + w], y_t[:, :w])

    nc.compile()
    return nc


def _get_kernel(C: int):
    if C not in _KERNEL_CACHE:
        _KERNEL_CACHE[C] = _build_kernel(C)
    return _KERNEL_CACHE[C]


def _route(xf, Wg, bg, top_k):
    """Replicate the reference gate: logits -> top-k -> softmax."""
    logits = xf.astype(np.float32) @ Wg.astype(np.float32) + bg.astype(np.float32)
    # jax.lax.top_k: values sorted descending, ties broken by lower index.
    order = np.argsort(-logits, axis=1, kind="stable")
    sel = order[:, :top_k]                                      # [T, K]
    vals = np.take_along_axis(logits, sel, axis=1)              # [T, K]
    vmax = vals.max(axis=1, keepdims=True)
    ex = np.exp((vals - vmax).astype(np.float32))
    w = ex / ex.sum(axis=1, keepdims=True)                      # [T, K]
    return sel, w.astype(np.float32)


def _plan(x, Wg, bg, top_k):
    """Routing plan: token indices + gate weight per expert, capacity C."""
    B, S, _ = x.shape
    xf = np.ascontiguousarray(x.reshape(B * S, D).astype(np.float32))
    sel, w = _route(xf, Wg, bg, top_k)
    idx_list, gate_list = [], []
    for e in range(E):
        hit = (sel == e)                    # [T, K]
        tok = np.nonzero(hit.any(axis=1))[0]
        kslot = hit[tok].argmax(axis=1)
        idx_list.append(tok)
        gate_list.append(w[tok, kslot])
    C = max(128, int(-(-max(len(t) for t in idx_list) // 64)) * 64)
    return xf, idx_list, gate_list, C


def _pack_inputs(xf, idx_list, C, W1, b1, W2, b2):
    xf_bf = xf.astype(ml_dtypes.bfloat16)
    in_maps = []
    for e in range(E):
        tok = idx_list[e]
        xe = np.zeros((C, D), dtype=ml_dtypes.bfloat16)
        xe[: len(tok)] = xf_bf[tok]
        in_maps.append(
            {
                "xT": np.ascontiguousarray(xe.reshape(C, KD, 128).transpose(2, 1, 0)),
                "w1": np.ascontiguousarray(
                    W1[e].astype(ml_dtypes.bfloat16)
                    .reshape(KD, 128, KF, 128).transpose(1, 2, 0, 3)
                    .reshape(128, KF * KD * 128)
                ),
                "w2": np.ascontiguousarray(
                    W2[e].astype(ml_dtypes.bfloat16)
                    .reshape(KF, 128, KD, 128).transpose(1, 2, 0, 3)
                    .reshape(128, KD * KF * 128)
                ),
                "b1": np.ascontiguousarray(b1[e].reshape(KF, 128).T.astype(np.float32)),
                "b2": np.ascontiguousarray(b2[e].reshape(KD, 128).T.astype(np.float32)),
            }
        )
    return in_maps


def _combine(results, idx_list, gate_list, C, T):
    out = np.zeros((T, D), dtype=np.float32)
    for e in range(E):
        tok = idx_list[e]
        if len(tok) == 0:
            continue
        y_pack = results[e]["yT"]                           # [KD, 128, C] f32
        ye = y_pack.transpose(2, 0, 1).reshape(C, D)[: len(tok)]
        out[tok] += gate_list[e][:, None] * ye
    return out


def kernel(x, W1, b1, W2, b2, Wg, bg, top_k):
    x = np.asarray(x)
    W1 = np.asarray(W1, dtype=np.float32)
    b1 = np.asarray(b1, dtype=np.float32)
    W2 = np.asarray(W2, dtype=np.float32)
    b2 = np.asarray(b2, dtype=np.float32)
    Wg = np.asarray(Wg, dtype=np.float32)
    bg = np.asarray(bg, dtype=np.float32)
    top_k = int(np.asarray(top_k))

    B, S, Din = x.shape
    xf, idx_list, gate_list, C = _plan(x, Wg, bg, top_k)
    in_maps = _pack_inputs(xf, idx_list, C, W1, b1, W2, b2)
    results = _run_device(C, in_maps)
    out = _combine(results, idx_list, gate_list, C, B * S)
    return out.reshape(B, S, Din).astype(np.float32)


def _run_device(C, in_maps):
    """Execute on the 8 cores, surviving the pool's transient device wedge.

    ~1 in 3 fresh sessions throws NRT_EXEC_UNIT_UNRECOVERABLE at NEFF load
    and the whole in-process jax/axon session stays poisoned, so after one
    in-process retry we re-run in a fresh subprocess (fresh device claim),
    which has always succeeded.
    """
    last_err = None
    for _ in range(2):
        try:
            return run_bass_kernel_spmd(_get_kernel(C), in_maps, list(range(E))).results
        except Exception as e:  # noqa: BLE001
            last_err = e
    import subprocess
    import sys
    import tempfile
    import os

    for _ in range(2):
        tmpd = tempfile.mkdtemp(prefix="moe_kernel_")
        in_path = os.path.join(tmpd, "in.npz")
        out_path = os.path.join(tmpd, "out.npz")
        payload = {"C": np.int64(C)}
        for i, m in enumerate(in_maps):
            for k, v in m.items():
                payload[f"{i}|{k}"] = (
                    v.view(np.uint16) if v.dtype == ml_dtypes.bfloat16 else v
                )
        np.savez(in_path, **payload)
        proc = subprocess.run(
            [sys.executable, os.path.abspath(__file__), in_path, out_path],
            capture_output=True,
            text=True,
        )
        if proc.returncode == 0 and os.path.exists(out_path):
            with np.load(out_path) as z:
                return [{"yT": z[f"{i}|yT"]} for i in range(E)]
        last_err = RuntimeError(
            f"subprocess retry failed (rc={proc.returncode}):\n{proc.stderr[-2000:]}"
        )
    raise last_err


def _subprocess_main(in_path, out_path):
    with np.load(in_path) as z:
        C = int(z["C"])
        in_maps = [{} for _ in range(E)]
        for key in z.files:
            if key == "C":
                continue
            i, name = key.split("|")
            v = z[key]
            if v.dtype == np.uint16:
                v = v.view(ml_dtypes.bfloat16)
            in_maps[int(i)][name] = v
    res = run_bass_kernel_spmd(_get_kernel(C), in_maps, list(range(E)))
    np.savez(out_path, **{f"{i}|yT": r["yT"] for i, r in enumerate(res.results)})


if __name__ == "__main__":
    import sys as _sys

    _subprocess_main(_sys.argv[1], _sys.argv[2])

